# revision 9
# baseline (speedup 1.0000x reference)
"""Trainium2 Bass kernel for a ViT-style transformer block (dense_transformer).

Sharding: pure data-parallel over batch. B=16 -> 2 batch items (1154 tokens)
per NeuronCore, weights replicated, no collectives.

v2 attention restructure:
  - Heads processed in pairs (2j, 2j+1) that live in partition halves 0-63 /
    64-127 of the same kc block of qT/kT.  Their S matmuls (K=64) are issued
    adjacently so bass's auto tile_position (0,0)/(64,0) lets the PE run them
    concurrently on distinct row-groups.
  - AV accumulates into one 3-bank PSUM pair tile; the two heads' 65-wide
    q-tail chunks share a bank as a single interleaved accumulation group
    (per-element has_written makes this correct; skip_group_check for sim).
  - Attention is ACT(exp)-bound; batch-0's out-proj + LN2 + transposes are
    interleaved into batch-1's attention window to fill the idle PE.
  - PSUM pools are phase-scoped so each phase gets the bank budget it needs.
"""

import numpy as np
import ml_dtypes

import concourse.bass as bass
import concourse.mybir as mybir
import concourse.tile as tile
from concourse import bacc
from concourse.bass_utils import run_bass_kernel_spmd
from concourse.masks import make_identity

F32 = mybir.dt.float32
BF16 = mybir.dt.bfloat16
AF = mybir.ActivationFunctionType
ALU = mybir.AluOpType

C = 1024
H = 16
HD = 64
HID = 4096
B = 16
N = 577
NCORES = 8
BPC = B // NCORES          # batch items per core = 2
T = BPC * N                # tokens per core = 1154
EPS = 1e-5
KC = C // 128              # 8 contraction chunks over C
MC = HID // 128            # 32 chunks over hidden

# per-batch token tiles: (offset, size, batch, idx) within the core's tokens
TOK_TILES = []
for b in range(BPC):
    for i in range(5):
        off = b * N + i * 128
        TOK_TILES.append((off, min(128, N - i * 128), b, i))

# free-dim chunks over the full 1154 tokens (for qk / fc1 rhs streaming)
TCHUNKS = [(0, 512), (512, 512), (1024, T - 1024)]
# free-dim chunks over one batch item's 577 queries
QCHUNKS = [(0, 512), (512, N - 512)]


def _bf(a):
    return np.ascontiguousarray(a.astype(ml_dtypes.bfloat16))


def _block_lhs(wt, n_m):
    """Blocked layout for M=c_out-orientation lhsT weights."""
    c_in = wt.shape[0]
    kc = c_in // 128
    return np.ascontiguousarray(
        wt.reshape(kc, 128, n_m, 128).transpose(2, 1, 0, 3).reshape(n_m, 128, c_in)
    )


def _build(use_bias):
    """Emit the single-core program (SPMD: all 8 cores run it)."""
    from contextlib import ExitStack

    nc = bacc.Bacc(None, target_bir_lowering=False, debug=False)

    x_d = nc.dram_tensor("x", [T, C], F32, kind="ExternalInput")
    wq_d = nc.dram_tensor("wq", [KC, 128, C], BF16, kind="ExternalInput")
    wk_d = nc.dram_tensor("wk", [KC, 128, C], BF16, kind="ExternalInput")
    wv_d = nc.dram_tensor("wv", [KC, 128, C], BF16, kind="ExternalInput")
    wp_d = nc.dram_tensor("wp", [KC, 128, C], BF16, kind="ExternalInput")
    w1_d = nc.dram_tensor("w1", [MC, 128, C], BF16, kind="ExternalInput")
    w2_d = nc.dram_tensor("w2", [MC, 128, C], BF16, kind="ExternalInput")
    bias_d = {}
    for nm, dim in (("bq", C), ("bk", C), ("bv", C), ("bp", C), ("b1", HID), ("b2", C)):
        if use_bias[nm]:
            bias_d[nm] = nc.dram_tensor(nm, [1, dim], BF16, kind="ExternalInput")
    out_d = nc.dram_tensor("out", [T, C], F32, kind="ExternalOutput")

    with tile.TileContext(nc) as tc, ExitStack() as top:
        const = top.enter_context(tc.tile_pool(name="const", bufs=1))
        p_out1 = top.enter_context(tc.tile_pool(name="p_out1", bufs=1))
        p_h2T = top.enter_context(tc.tile_pool(name="p_h2T", bufs=1))

        ident = const.tile([128, 128], BF16)
        make_identity(nc, ident)
        ones_r = const.tile([1, 512], BF16)
        nc.vector.memset(ones_r, 1.0)
        ones_m = const.tile([1, 128], BF16)
        nc.vector.memset(ones_m, 1.0)
        eps_t = const.tile([128, 1], F32)
        nc.vector.memset(eps_t, EPS)
        bias_sb = {}
        for nm, t in bias_d.items():
            bt = const.tile([1, t.shape[1]], BF16, name=f"b_{nm}")
            nc.sync.dma_start(out=bt, in_=t[:])
            bias_sb[nm] = bt

        out1 = p_out1.tile([128, len(TOK_TILES), C], BF16)
        h2T = p_h2T.tile([128, KC, T], BF16, name="h2T")

        # first use of the gpsimd/DVE custom-op libraries costs a ~7us
        # library load; pay it here, overlapped with the x DMAs
        wu_a = const.tile([1, 8], F32, name="wu_a")
        nc.vector.memset(wu_a, 1.0)
        wu_b = const.tile([8, 8], F32, name="wu_b")
        nc.vector.reciprocal_approx_fast(out=wu_a, in_=wu_a)
        nc.gpsimd.partition_broadcast(wu_b, wu_a)

        def ln_tile(pool, x_t, sz, lab):
            """Standardize one token tile -> bf16 h tile (g/b folded into
            the downstream weights host-side)."""
            st = pool.tile([128, 2, 6], F32, name=f"st{lab}")
            for c2 in range(2):
                nc.vector.bn_stats(
                    out=st[:sz, c2, :], in_=x_t[:sz, c2 * 512 : (c2 + 1) * 512]
                )
            mv = pool.tile([128, 2], F32, name=f"mv{lab}")
            nc.vector.bn_aggr(out=mv[:sz], in_=st[:sz])
            sd = pool.tile([128, 1], F32, name=f"sd{lab}")
            nc.scalar.activation(
                out=sd[:sz], in_=mv[:sz, 1:2], func=AF.Sqrt, bias=eps_t[:sz]
            )
            rstd = pool.tile([128, 1], F32, name=f"rs{lab}")
            nc.vector.reciprocal(out=rstd[:sz], in_=sd[:sz])
            h_t = pool.tile([128, C], BF16, name=f"h{lab}")
            nc.vector.tensor_scalar(
                out=h_t[:sz],
                in0=x_t[:sz],
                scalar1=mv[:sz, 0:1],
                scalar2=rstd[:sz],
                op0=ALU.subtract,
                op1=ALU.mult,
            )
            return h_t

        def transpose_into(ps_pool, h_t, sz, dstT, off, tag="mm", evict="scalar",
                           bufs=2):
            for kc in range(KC):
                ptr = ps_pool.tile([128, 512], BF16, tag=tag, name="tr", bufs=bufs)
                nc.tensor.transpose(
                    ptr[:, :sz], h_t[:sz, kc * 128 : (kc + 1) * 128], ident[:sz, :sz]
                )
                if evict == "scalar":
                    nc.scalar.copy(out=dstT[:, kc, off : off + sz], in_=ptr[:, :sz])
                else:
                    nc.vector.tensor_copy(
                        out=dstT[:, kc, off : off + sz], in_=ptr[:, :sz]
                    )

        # ---------------- phase 1: attention ----------------
        with ExitStack() as ph1:
            p_wres = ph1.enter_context(tc.tile_pool(name="p_wres", bufs=1))
            p_oT = ph1.enter_context(tc.tile_pool(name="p_oT", bufs=1))
            p_dead = ph1.enter_context(tc.tile_pool(name="p_dead", bufs=1))
            p_e = ph1.enter_context(tc.tile_pool(name="p_e", bufs=1))

            wv_sb = p_wres.tile([128, KC, C], BF16, name="wv_sb")
            wp_sb = p_wres.tile([128, KC, C], BF16, name="wp_sb")
            for kc in range(KC):
                nc.sync.dma_start(out=wv_sb[:, kc, :], in_=wv_d[kc])
                nc.sync.dma_start(out=wp_sb[:, kc, :], in_=wp_d[kc])
            oTb = [p_oT.tile([128, KC, N], BF16, name=f"oT{b}")
                   for b in range(BPC)]
            qT = p_dead.tile([128, KC, T], BF16, name="qT")
            kT = p_dead.tile([128, KC, T], BF16, name="kT")
            v_att = p_dead.tile([128, len(TOK_TILES), H * 65], BF16, name="v_att")

            # ---- phase 1a: LN1 + v + q/k projections ----
            with ExitStack() as ph1a:
                psA = ph1a.enter_context(
                    tc.tile_pool(name="psA", bufs=1, space="PSUM")
                )
                p_x = ph1a.enter_context(tc.tile_pool(name="p_x", bufs=3))
                p_ln = ph1a.enter_context(tc.tile_pool(name="p_ln", bufs=2))
                p_w = ph1a.enter_context(tc.tile_pool(name="p_w", bufs=3))
                p_hT = ph1a.enter_context(tc.tile_pool(name="p_hT", bufs=1))
                hT = p_hT.tile([128, KC, T], BF16, name="hT")

                def psum_mm(tag="mm"):
                    return psA.tile([128, 512], F32, tag=tag, name=tag, bufs=3)

                # LN1 + transpose to feature-major
                for ti, (off, sz, _, _) in enumerate(TOK_TILES):
                    x_t = p_x.tile([128, C], F32, name="x_t")
                    nc.sync.dma_start(out=x_t[:sz], in_=x_d[off : off + sz, :])
                    h_t = ln_tile(p_ln, x_t, sz, "1")
                    transpose_into(psA, h_t, sz, hT, off, tag="tr")

                # v first (its DVE evictions must complete before attention)
                nc.vector.memset(
                    v_att.rearrange("p t (h e) -> p t h e", e=65)[:, :, :, 64:65],
                    1.0,
                )
                for ti, (off, sz, _, _) in enumerate(TOK_TILES):
                    for ci in range(2):
                        pm = psum_mm()
                        for kc in range(KC):
                            nc.tensor.matmul(
                                pm[:sz],
                                hT[:, kc, off : off + sz],
                                wv_sb[:, kc, ci * 512 : (ci + 1) * 512],
                                start=(kc == 0),
                                stop=(kc == KC - 1 and "bv" not in bias_sb),
                            )
                        if "bv" in bias_sb:
                            nc.tensor.matmul(
                                pm[:sz],
                                ones_m[0:1, :sz],
                                bias_sb["bv"][0:1, ci * 512 : (ci + 1) * 512],
                                start=False,
                                stop=True,
                            )
                        dst = v_att.rearrange("p t (h e) -> p t h e", e=65)[
                            :sz, ti, ci * 8 : (ci + 1) * 8, 0:64
                        ]
                        nc.vector.tensor_copy(
                            out=dst, in_=pm[:sz].rearrange("p (h e) -> p h e", e=64)
                        )

                # q^T / k^T (feature-major)
                for wd, bnm, outT in ((wq_d, "bq", qT), (wk_d, "bk", kT)):
                    for m in range(KC):
                        w_t = p_w.tile([128, C], BF16, name="w_t")
                        nc.sync.dma_start(out=w_t, in_=wd[m])
                        for c0, csz in TCHUNKS:
                            pm = psum_mm()
                            for kc in range(KC):
                                nc.tensor.matmul(
                                    pm[:, :csz],
                                    w_t[:, kc * 128 : (kc + 1) * 128],
                                    hT[:, kc, c0 : c0 + csz],
                                    start=(kc == 0),
                                    stop=(kc == KC - 1 and bnm not in bias_sb),
                                )
                            if bnm in bias_sb:
                                nc.tensor.matmul(
                                    pm[:, :csz],
                                    bias_sb[bnm][0:1, m * 128 : (m + 1) * 128],
                                    ones_r[0:1, :csz],
                                    start=False,
                                    stop=True,
                                )
                            # alternate eviction engine to balance ACT/DVE
                            if m & 1:
                                nc.scalar.copy(
                                    out=outT[:, m, c0 : c0 + csz], in_=pm[:, :csz]
                                )
                            else:
                                nc.vector.tensor_copy(
                                    out=outT[:, m, c0 : c0 + csz], in_=pm[:, :csz]
                                )

            # ---- phase 1b: attention core, b0 plain, b1 + b0-proj fused ----
            with ExitStack() as ph1b:
                psB = ph1b.enter_context(
                    tc.tile_pool(name="psB", bufs=1, space="PSUM")
                )
                p_s = ph1b.enter_context(tc.tile_pool(name="p_s", bufs=2))
                p_xr = ph1b.enter_context(tc.tile_pool(name="p_xr", bufs=6))
                p_ln2 = ph1b.enter_context(tc.tile_pool(name="p_ln2", bufs=2))

                pending = []

                def flush_pending():
                    while pending:
                        dst, r_bc_prev = pending.pop(0)
                        nc.vector.tensor_tensor(
                            out=dst, in0=dst, in1=r_bc_prev, op=ALU.mult
                        )

                def proj_chunk(ti, ci):
                    """out-proj chunk + residual-1 into out1 (PE+DVE work)."""
                    off, sz, bt, _ = TOK_TILES[ti]
                    pm = psB.tile([128, 512], F32, tag="mm", name="pj", bufs=1)
                    for kc in range(KC):
                        nc.tensor.matmul(
                            pm[:sz],
                            oTb[bt][:, kc, off - bt * N : off - bt * N + sz],
                            wp_sb[:, kc, ci * 512 : (ci + 1) * 512],
                            start=(kc == 0),
                            stop=(kc == KC - 1 and "bp" not in bias_sb),
                        )
                    if "bp" in bias_sb:
                        nc.tensor.matmul(
                            pm[:sz],
                            ones_m[0:1, :sz],
                            bias_sb["bp"][0:1, ci * 512 : (ci + 1) * 512],
                            start=False,
                            stop=True,
                        )
                    x_t2 = p_xr.tile([128, 512], F32, name="x_t2")
                    nc.sync.dma_start(
                        out=x_t2[:sz],
                        in_=x_d[off : off + sz, ci * 512 : (ci + 1) * 512],
                    )
                    nc.vector.scalar_tensor_tensor(
                        out=out1[:sz, ti, ci * 512 : (ci + 1) * 512],
                        in0=pm[:sz],
                        scalar=1.0,
                        in1=x_t2[:sz],
                        op0=ALU.mult,
                        op1=ALU.add,
                    )

                def attn_pair(b, j, fillers):
                    """One head pair (2j, 2j+1); fillers: list of closures to
                    interleave for PE occupancy while ACT grinds the exps."""
                    sb0 = b * N
                    # AV pair accumulator: 3 banks.
                    # cols [0:512]    h0 q-chunk0   (bank 0)
                    # cols [512:577]  h0 q-chunk1   (bank 1, shared group)
                    # cols [577:642]  h1 q-chunk1   (bank 1, shared group)
                    # cols [1024:1536] h1 q-chunk0  (bank 2)
                    av = psB.tile([65, 1536], F32, tag="av", name="av", bufs=1)

                    def av_pair(e_kt, kt, ksz):
                        for hi in range(2):
                            h = 2 * j + hi
                            vt = v_att[:ksz, b * 5 + kt, h * 65 : (h + 1) * 65]
                            # q-chunk0 -> own bank
                            c0off = hi * 1024
                            nc.tensor.matmul(
                                av[:, c0off : c0off + 512],
                                vt,
                                e_kt[hi][:ksz, 0:512],
                                start=(kt == 0),
                                stop=(kt == 4),
                            )
                            # q-chunk1 -> shared bank-1 group
                            c1off = 512 + hi * 65
                            nc.tensor.matmul(
                                av[:, c1off : c1off + 65],
                                vt,
                                e_kt[hi][:ksz, 512:577],
                                start=(kt == 0 and hi == 0),
                                stop=(kt == 4 and hi == 1),
                                skip_group_check=True,
                            )

                    # software-pipelined by one kt: S/exp(kt+1) issue ahead of
                    # AV(kt) so the PE never queues behind the ACT exps
                    avq = []
                    for kt in range(5):
                        koff = sb0 + kt * 128
                        ksz = min(128, N - kt * 128)
                        # chunk-major issue order: the two heads' 512-wide S
                        # matmuls are adjacent in the PE queue and target
                        # disjoint row-groups (0-63 / 64-127), so they run
                        # concurrently on the array
                        ps_ss = [
                            psB.tile([128, N], F32, tag="s", name="ps_s", bufs=2)
                            for _ in range(2)
                        ]
                        for q0, qsz in QCHUNKS:
                            for hi in range(2):
                                po = hi * 64
                                nc.tensor.matmul(
                                    ps_ss[hi][:ksz, q0 : q0 + qsz],
                                    kT[po : po + 64, j, koff : koff + ksz],
                                    qT[po : po + 64, j, sb0 + q0 : sb0 + q0 + qsz],
                                    start=True,
                                    stop=True,
                                )
                        e_kt = []
                        for hi in range(2):
                            e_t = p_e.tile([128, N], BF16, tag="E", name="E", bufs=4)
                            nc.scalar.activation(
                                out=e_t[:ksz], in_=ps_ss[hi][:ksz],
                                func=AF.Exp, scale=0.125,
                            )
                            e_kt.append(e_t)
                        avq.append((e_kt, kt, ksz))
                        if kt >= 1:
                            av_pair(*avq.pop(0))
                        if kt == 2 and fillers:
                            fillers.pop(0)()
                    av_pair(*avq.pop(0))
                    # evict unnormalized o + sums, then normalize (deferred)
                    for hi in range(2):
                        po = hi * 64
                        c0off = hi * 1024
                        c1off = 512 + hi * 65
                        nc.vector.tensor_copy(
                            out=oTb[b][po : po + 64, j, 0:512],
                            in_=av[0:64, c0off : c0off + 512],
                        )
                        nc.vector.tensor_copy(
                            out=oTb[b][po : po + 64, j, 512:577],
                            in_=av[0:64, c1off : c1off + 65],
                        )
                        s_sb = p_s.tile([1, N], F32, name="s_sb", bufs=3)
                        nc.vector.tensor_copy(
                            out=s_sb[:, 0:512], in_=av[64:65, c0off : c0off + 512]
                        )
                        nc.vector.tensor_copy(
                            out=s_sb[:, 512:577], in_=av[64:65, c1off : c1off + 65]
                        )
                        r_sb = p_s.tile([1, N], F32, name="r_sb", bufs=3)
                        nc.vector.reciprocal_approx_fast(out=r_sb, in_=s_sb)
                        r_bc = p_s.tile([128, N], F32, name="r_bc", bufs=3)
                        nc.gpsimd.partition_broadcast(r_bc, r_sb)
                        flush_pending()
                        pending.append(
                            (oTb[b][po : po + 64, j, :], r_bc[po : po + 64])
                        )

                # b0 attention, no fillers
                for j in range(KC):
                    attn_pair(0, j, [])
                flush_pending()

                # b1 attention with b0 proj interleaved (PE-heavy pieces;
                # LN2 stays out of this window: its Sqrt would force an ACT
                # table switch away from Exp, ~2.7us each way)
                work = []
                for ti in range(5):
                    work.append(lambda ti=ti: proj_chunk(ti, 0))
                    work.append(lambda ti=ti: proj_chunk(ti, 1))
                for j in range(KC):
                    attn_pair(1, j, work)
                flush_pending()
                while work:
                    work.pop(0)()

            # ---- phase 1c: b1 proj + LN2 + transposes (all tiles) ----
            with ExitStack() as ph1c:
                psD = ph1c.enter_context(
                    tc.tile_pool(name="psD", bufs=1, space="PSUM")
                )
                p_xr2 = ph1c.enter_context(tc.tile_pool(name="p_xr2", bufs=4))
                p_ln2b = ph1c.enter_context(tc.tile_pool(name="p_ln2b", bufs=2))
                for ti in range(5, 10):
                    off, sz, bt, _ = TOK_TILES[ti]
                    for ci in range(2):
                        pm = psD.tile([128, 512], F32, tag="mm", name="pj2", bufs=2)
                        for kc in range(KC):
                            nc.tensor.matmul(
                                pm[:sz],
                                oTb[bt][:, kc, off - bt * N : off - bt * N + sz],
                                wp_sb[:, kc, ci * 512 : (ci + 1) * 512],
                                start=(kc == 0),
                                stop=(kc == KC - 1 and "bp" not in bias_sb),
                            )
                        if "bp" in bias_sb:
                            nc.tensor.matmul(
                                pm[:sz],
                                ones_m[0:1, :sz],
                                bias_sb["bp"][0:1, ci * 512 : (ci + 1) * 512],
                                start=False,
                                stop=True,
                            )
                        x_t2 = p_xr2.tile([128, 512], F32, name="x_t2")
                        nc.sync.dma_start(
                            out=x_t2[:sz],
                            in_=x_d[off : off + sz, ci * 512 : (ci + 1) * 512],
                        )
                        nc.vector.scalar_tensor_tensor(
                            out=out1[:sz, ti, ci * 512 : (ci + 1) * 512],
                            in0=pm[:sz],
                            scalar=1.0,
                            in1=x_t2[:sz],
                            op0=ALU.mult,
                            op1=ALU.add,
                        )
                # LN2 + transpose for every token tile (b0's out1 is ready;
                # overlaps the b1 proj above via engine parallelism)
                for ti, (off, sz, _, _) in enumerate(TOK_TILES):
                    h_t2 = ln_tile(p_ln2b, out1[:, ti, :], sz, "2")
                    transpose_into(psD, h_t2, sz, h2T, off, tag="tr")

        # ---------------- phase 2: MLP ----------------
        with ExitStack() as ph2:
            psC = ph2.enter_context(tc.tile_pool(name="psC", bufs=1, space="PSUM"))
            p_gu = ph2.enter_context(tc.tile_pool(name="p_gu", bufs=1))
            p_w2r = ph2.enter_context(tc.tile_pool(name="p_w2r", bufs=1))
            p_w1s = ph2.enter_context(tc.tile_pool(name="p_w1s", bufs=3))
            p_sg = ph2.enter_context(tc.tile_pool(name="p_sg", bufs=2))
            p_res = ph2.enter_context(tc.tile_pool(name="p_res", bufs=4))

            guT = p_gu.tile([128, MC, T], BF16, name="guT")
            w2sb = p_w2r.tile([128, MC, C], BF16, name="w2sb")

            # fc1 + fast-gelu; W2 prefetch rides the gpsimd DGE queue
            for m in range(MC):
                w_t = p_w1s.tile([128, C], BF16, name="w1_t")
                nc.sync.dma_start(out=w_t, in_=w1_d[m])
                nc.gpsimd.dma_start(out=w2sb[:, m, :], in_=w2_d[m])
                for c0, csz in TCHUNKS:
                    pm = psC.tile(
                        [128, 512], F32,
                        tag="mm" if (m & 1) == 0 else "tr",
                        name="f1", bufs=2,
                    )
                    for kc in range(KC):
                        nc.tensor.matmul(
                            pm[:, :csz],
                            w_t[:, kc * 128 : (kc + 1) * 128],
                            h2T[:, kc, c0 : c0 + csz],
                            start=(kc == 0),
                            stop=(kc == KC - 1 and "b1" not in bias_sb),
                        )
                    if "b1" in bias_sb:
                        nc.tensor.matmul(
                            pm[:, :csz],
                            bias_sb["b1"][0:1, m * 128 : (m + 1) * 128],
                            ones_r[0:1, :csz],
                            start=False,
                            stop=True,
                        )
                    sg = p_sg.tile([128, 512], F32, name="sg")
                    nc.scalar.activation(
                        out=sg[:, :csz], in_=pm[:, :csz], func=AF.Sigmoid, scale=1.702
                    )
                    nc.vector.tensor_tensor(
                        out=guT[:, m, c0 : c0 + csz],
                        in0=pm[:, :csz],
                        in1=sg[:, :csz],
                        op=ALU.mult,
                    )

            # fc2 + residual-2, straight to DRAM
            for ti, (off, sz, _, _) in enumerate(TOK_TILES):
                pms = [
                    psC.tile([128, 512], F32, tag="mm", name="f2a", bufs=2),
                    psC.tile([128, N], F32, tag="wide", name="f2b", bufs=2),
                ]
                for hc in range(MC):
                    for half in range(2):
                        nc.tensor.matmul(
                            pms[half][:sz, :512],
                            guT[:, hc, off : off + sz],
                            w2sb[:, hc, half * 512 : half * 512 + 512],
                            start=(hc == 0),
                            stop=(hc == MC - 1 and "b2" not in bias_sb),
                        )
                for half in range(2):
                    h0 = half * 512
                    if "b2" in bias_sb:
                        nc.tensor.matmul(
                            pms[half][:sz, :512],
                            ones_m[0:1, :sz],
                            bias_sb["b2"][0:1, h0 : h0 + 512],
                            start=False,
                            stop=True,
                        )
                    res = p_res.tile([128, 512], F32, name="res")
                    nc.vector.scalar_tensor_tensor(
                        out=res[:sz],
                        in0=pms[half][:sz, :512],
                        scalar=1.0,
                        in1=out1[:sz, ti, h0 : h0 + 512],
                        op0=ALU.mult,
                        op1=ALU.add,
                    )
                    nc.sync.dma_start(
                        out=out_d[off : off + sz, h0 : h0 + 512], in_=res[:sz]
                    )

    nc.compile()
    return nc


def _prepare(inputs):
    """Host-side: fold norms into weights, transpose/block, shard x."""
    f = lambda k: np.asarray(inputs[k], dtype=np.float32)
    x = f("x")
    g1, b1ln = f("norm1_g"), f("norm1_b")
    g2, b2ln = f("norm2_g"), f("norm2_b")
    Wq, Wk, Wv, Wp = f("Wq"), f("Wk"), f("Wv"), f("Wp")
    W1, W2 = f("W1"), f("W2")
    bp, b1, b2 = f("bp"), f("b1"), f("b2")

    wq = _block_lhs((Wq * g1[None, :]).T, KC)
    wk = _block_lhs((Wk * g1[None, :]).T, KC)
    wv = np.ascontiguousarray((Wv * g1[None, :]).T.reshape(KC, 128, C))
    wp = np.ascontiguousarray(Wp.T.reshape(KC, 128, C))
    w1 = _block_lhs((W1 * g2[None, :]).T, MC)
    w2 = np.ascontiguousarray(W2.T.reshape(MC, 128, C))

    biases = {
        "bq": Wq @ b1ln,
        "bk": Wk @ b1ln,
        "bv": Wv @ b1ln,
        "bp": bp,
        "b1": W1 @ b2ln + b1,
        "b2": b2,
    }
    use_bias = {k: bool(np.any(v != 0)) for k, v in biases.items()}

    weights = {
        "wq": _bf(wq),
        "wk": _bf(wk),
        "wv": _bf(wv),
        "wp": _bf(wp),
        "w1": _bf(w1),
        "w2": _bf(w2),
    }
    for k, v in biases.items():
        if use_bias[k]:
            weights[k] = _bf(v.reshape(1, -1))

    xs = x.reshape(NCORES, T, C)
    in_maps = [dict(weights, x=np.ascontiguousarray(xs[i])) for i in range(NCORES)]
    return in_maps, use_bias


_CACHE = {}


def _get_program(use_bias):
    key = tuple(sorted(use_bias.items()))
    if key not in _CACHE:
        _CACHE[key] = _build(use_bias)
    return _CACHE[key]


def run(inputs, trace=False, **kw):
    in_maps, use_bias = _prepare(inputs)
    nc = _get_program(use_bias)
    res = run_bass_kernel_spmd(
        nc, in_maps, core_ids=list(range(NCORES)), trace=trace, **kw
    )
    out = np.concatenate([res.results[i]["out"] for i in range(NCORES)], axis=0)
    return np.ascontiguousarray(out.astype(np.float32)), res


def kernel(**inputs):
    out, _ = run(inputs, trace=False)
    return out


# revision 11
# speedup vs baseline: 1.0315x; 1.0315x over previous
"""Trainium2 Bass kernel for a ViT-style transformer block (dense_transformer).

Sharding: pure data-parallel over batch. B=16 -> 2 batch items (1154 tokens)
per NeuronCore, weights replicated, no collectives.

v2 attention restructure:
  - Heads processed in pairs (2j, 2j+1) that live in partition halves 0-63 /
    64-127 of the same kc block of qT/kT.  Their S matmuls (K=64) are issued
    adjacently so bass's auto tile_position (0,0)/(64,0) lets the PE run them
    concurrently on distinct row-groups.
  - AV accumulates into one 3-bank PSUM pair tile; the two heads' 65-wide
    q-tail chunks share a bank as a single interleaved accumulation group
    (per-element has_written makes this correct; skip_group_check for sim).
  - Attention is ACT(exp)-bound; batch-0's out-proj + LN2 + transposes are
    interleaved into batch-1's attention window to fill the idle PE.
  - PSUM pools are phase-scoped so each phase gets the bank budget it needs.
"""

import numpy as np
import ml_dtypes

import concourse.bass as bass
import concourse.mybir as mybir
import concourse.tile as tile
from concourse import bacc
from concourse.bass_utils import run_bass_kernel_spmd
from concourse.masks import make_identity

F32 = mybir.dt.float32
BF16 = mybir.dt.bfloat16
AF = mybir.ActivationFunctionType
ALU = mybir.AluOpType

C = 1024
H = 16
HD = 64
HID = 4096
B = 16
N = 577
NCORES = 8
BPC = B // NCORES          # batch items per core = 2
T = BPC * N                # tokens per core = 1154
EPS = 1e-5
KC = C // 128              # 8 contraction chunks over C
MC = HID // 128            # 32 chunks over hidden

# per-batch token tiles: (offset, size, batch, idx) within the core's tokens
TOK_TILES = []
for b in range(BPC):
    for i in range(5):
        off = b * N + i * 128
        TOK_TILES.append((off, min(128, N - i * 128), b, i))

# free-dim chunks over the full 1154 tokens (for qk / fc1 rhs streaming)
TCHUNKS = [(0, 512), (512, 512), (1024, T - 1024)]
# free-dim chunks over one batch item's 577 queries
QCHUNKS = [(0, 512), (512, N - 512)]


def _bf(a):
    return np.ascontiguousarray(a.astype(ml_dtypes.bfloat16))


def _block_lhs(wt, n_m):
    """Blocked layout for M=c_out-orientation lhsT weights."""
    c_in = wt.shape[0]
    kc = c_in // 128
    return np.ascontiguousarray(
        wt.reshape(kc, 128, n_m, 128).transpose(2, 1, 0, 3).reshape(n_m, 128, c_in)
    )


def _build(use_bias):
    """Emit the single-core program (SPMD: all 8 cores run it)."""
    from contextlib import ExitStack

    nc = bacc.Bacc(None, target_bir_lowering=False, debug=False)

    x_d = nc.dram_tensor("x", [T, C], F32, kind="ExternalInput")
    wq_d = nc.dram_tensor("wq", [KC, 128, C], BF16, kind="ExternalInput")
    wk_d = nc.dram_tensor("wk", [KC, 128, C], BF16, kind="ExternalInput")
    wv_d = nc.dram_tensor("wv", [KC, 128, C], BF16, kind="ExternalInput")
    wp_d = nc.dram_tensor("wp", [KC, 128, C], BF16, kind="ExternalInput")
    w1_d = nc.dram_tensor("w1", [MC, 128, C], BF16, kind="ExternalInput")
    w2_d = nc.dram_tensor("w2", [MC, 128, C], BF16, kind="ExternalInput")
    bias_d = {}
    for nm, dim in (("bq", C), ("bk", C), ("bv", C), ("bp", C), ("b1", HID), ("b2", C)):
        if use_bias[nm]:
            bias_d[nm] = nc.dram_tensor(nm, [1, dim], BF16, kind="ExternalInput")
    out_d = nc.dram_tensor("out", [T, C], F32, kind="ExternalOutput")

    with tile.TileContext(nc) as tc, ExitStack() as top:
        const = top.enter_context(tc.tile_pool(name="const", bufs=1))
        p_out1 = top.enter_context(tc.tile_pool(name="p_out1", bufs=1))
        p_h2T = top.enter_context(tc.tile_pool(name="p_h2T", bufs=1))

        ident = const.tile([128, 128], BF16)
        make_identity(nc, ident)
        ones_r = const.tile([1, 512], BF16)
        nc.vector.memset(ones_r, 1.0)
        ones_m = const.tile([1, 128], BF16)
        nc.vector.memset(ones_m, 1.0)
        eps_t = const.tile([128, 1], F32)
        nc.vector.memset(eps_t, EPS)
        bias_sb = {}
        for nm, t in bias_d.items():
            bt = const.tile([1, t.shape[1]], BF16, name=f"b_{nm}")
            nc.sync.dma_start(out=bt, in_=t[:])
            bias_sb[nm] = bt

        out1 = p_out1.tile([128, len(TOK_TILES), C], BF16)
        h2T = p_h2T.tile([128, KC, T], BF16, name="h2T")

        # first use of the gpsimd/DVE custom-op libraries costs a ~7us
        # library load; pay it here, overlapped with the x DMAs
        wu_a = const.tile([1, 8], F32, name="wu_a")
        nc.vector.memset(wu_a, 1.0)
        wu_b = const.tile([8, 8], F32, name="wu_b")
        nc.vector.reciprocal_approx_fast(out=wu_a, in_=wu_a)
        nc.gpsimd.partition_broadcast(wu_b, wu_a)

        def ln_tile(pool, x_t, sz, lab):
            """Standardize one token tile -> bf16 h tile (g/b folded into
            the downstream weights host-side)."""
            st = pool.tile([128, 2, 6], F32, name=f"st{lab}")
            for c2 in range(2):
                nc.vector.bn_stats(
                    out=st[:sz, c2, :], in_=x_t[:sz, c2 * 512 : (c2 + 1) * 512]
                )
            mv = pool.tile([128, 2], F32, name=f"mv{lab}")
            nc.vector.bn_aggr(out=mv[:sz], in_=st[:sz])
            sd = pool.tile([128, 1], F32, name=f"sd{lab}")
            nc.scalar.activation(
                out=sd[:sz], in_=mv[:sz, 1:2], func=AF.Sqrt, bias=eps_t[:sz]
            )
            rstd = pool.tile([128, 1], F32, name=f"rs{lab}")
            nc.vector.reciprocal(out=rstd[:sz], in_=sd[:sz])
            h_t = pool.tile([128, C], BF16, name=f"h{lab}")
            nc.vector.tensor_scalar(
                out=h_t[:sz],
                in0=x_t[:sz],
                scalar1=mv[:sz, 0:1],
                scalar2=rstd[:sz],
                op0=ALU.subtract,
                op1=ALU.mult,
            )
            return h_t

        def transpose_into(ps_pool, h_t, sz, dstT, off, tag="mm", evict="scalar",
                           bufs=2):
            for kc in range(KC):
                ptr = ps_pool.tile([128, 512], BF16, tag=tag, name="tr", bufs=bufs)
                nc.tensor.transpose(
                    ptr[:, :sz], h_t[:sz, kc * 128 : (kc + 1) * 128], ident[:sz, :sz]
                )
                if evict == "scalar":
                    nc.scalar.copy(out=dstT[:, kc, off : off + sz], in_=ptr[:, :sz])
                else:
                    nc.vector.tensor_copy(
                        out=dstT[:, kc, off : off + sz], in_=ptr[:, :sz]
                    )

        # ---------------- phase 1: attention ----------------
        with ExitStack() as ph1:
            p_wres = ph1.enter_context(tc.tile_pool(name="p_wres", bufs=1))
            p_oT = ph1.enter_context(tc.tile_pool(name="p_oT", bufs=1))
            p_dead = ph1.enter_context(tc.tile_pool(name="p_dead", bufs=1))
            p_e = ph1.enter_context(tc.tile_pool(name="p_e", bufs=1))

            wv_sb = p_wres.tile([128, KC, C], BF16, name="wv_sb")
            wp_sb = p_wres.tile([128, KC, C], BF16, name="wp_sb")
            for kc in range(KC):
                nc.sync.dma_start(out=wv_sb[:, kc, :], in_=wv_d[kc])
                nc.sync.dma_start(out=wp_sb[:, kc, :], in_=wp_d[kc])
            oTb = [p_oT.tile([128, KC, N], BF16, name=f"oT{b}")
                   for b in range(BPC)]
            qT = p_dead.tile([128, KC, T], BF16, name="qT")
            kT = p_dead.tile([128, KC, T], BF16, name="kT")
            v_att = p_dead.tile([128, len(TOK_TILES), H * 65], BF16, name="v_att")

            # ---- phase 1a: LN1 + v + q/k projections ----
            with ExitStack() as ph1a:
                psA = ph1a.enter_context(
                    tc.tile_pool(name="psA", bufs=1, space="PSUM")
                )
                p_x = ph1a.enter_context(tc.tile_pool(name="p_x", bufs=3))
                p_ln = ph1a.enter_context(tc.tile_pool(name="p_ln", bufs=2))
                p_w = ph1a.enter_context(tc.tile_pool(name="p_w", bufs=3))
                p_hT = ph1a.enter_context(tc.tile_pool(name="p_hT", bufs=1))
                hT = p_hT.tile([128, KC, T], BF16, name="hT")

                def psum_mm(tag="mm"):
                    return psA.tile([128, 512], F32, tag=tag, name=tag, bufs=3)

                # LN1 + transpose to feature-major
                for ti, (off, sz, _, _) in enumerate(TOK_TILES):
                    x_t = p_x.tile([128, C], F32, name="x_t")
                    nc.sync.dma_start(out=x_t[:sz], in_=x_d[off : off + sz, :])
                    h_t = ln_tile(p_ln, x_t, sz, "1")
                    transpose_into(psA, h_t, sz, hT, off, tag="tr")

                # v first (its DVE evictions must complete before attention)
                nc.vector.memset(
                    v_att.rearrange("p t (h e) -> p t h e", e=65)[:, :, :, 64:65],
                    1.0,
                )
                for ti, (off, sz, _, _) in enumerate(TOK_TILES):
                    for ci in range(2):
                        pm = psum_mm()
                        for kc in range(KC):
                            nc.tensor.matmul(
                                pm[:sz],
                                hT[:, kc, off : off + sz],
                                wv_sb[:, kc, ci * 512 : (ci + 1) * 512],
                                start=(kc == 0),
                                stop=(kc == KC - 1 and "bv" not in bias_sb),
                            )
                        if "bv" in bias_sb:
                            nc.tensor.matmul(
                                pm[:sz],
                                ones_m[0:1, :sz],
                                bias_sb["bv"][0:1, ci * 512 : (ci + 1) * 512],
                                start=False,
                                stop=True,
                            )
                        dst = v_att.rearrange("p t (h e) -> p t h e", e=65)[
                            :sz, ti, ci * 8 : (ci + 1) * 8, 0:64
                        ]
                        nc.vector.tensor_copy(
                            out=dst, in_=pm[:sz].rearrange("p (h e) -> p h e", e=64)
                        )

                # q^T / k^T (feature-major)
                for wd, bnm, outT in ((wq_d, "bq", qT), (wk_d, "bk", kT)):
                    for m in range(KC):
                        w_t = p_w.tile([128, C], BF16, name="w_t")
                        nc.sync.dma_start(out=w_t, in_=wd[m])
                        for c0, csz in TCHUNKS:
                            pm = psum_mm()
                            for kc in range(KC):
                                nc.tensor.matmul(
                                    pm[:, :csz],
                                    w_t[:, kc * 128 : (kc + 1) * 128],
                                    hT[:, kc, c0 : c0 + csz],
                                    start=(kc == 0),
                                    stop=(kc == KC - 1 and bnm not in bias_sb),
                                )
                            if bnm in bias_sb:
                                nc.tensor.matmul(
                                    pm[:, :csz],
                                    bias_sb[bnm][0:1, m * 128 : (m + 1) * 128],
                                    ones_r[0:1, :csz],
                                    start=False,
                                    stop=True,
                                )
                            # alternate eviction engine to balance ACT/DVE
                            if m & 1:
                                nc.scalar.copy(
                                    out=outT[:, m, c0 : c0 + csz], in_=pm[:, :csz]
                                )
                            else:
                                nc.vector.tensor_copy(
                                    out=outT[:, m, c0 : c0 + csz], in_=pm[:, :csz]
                                )

            # ---- phase 1b: attention core, b0 plain, b1 + b0-proj fused ----
            with ExitStack() as ph1b:
                psB = ph1b.enter_context(
                    tc.tile_pool(name="psB", bufs=1, space="PSUM")
                )
                p_s = ph1b.enter_context(tc.tile_pool(name="p_s", bufs=2))
                p_xr = ph1b.enter_context(tc.tile_pool(name="p_xr", bufs=6))
                p_ln2 = ph1b.enter_context(tc.tile_pool(name="p_ln2", bufs=2))

                pending = []

                def flush_pending():
                    while pending:
                        dst, r_bc_prev = pending.pop(0)
                        nc.vector.tensor_tensor(
                            out=dst, in0=dst, in1=r_bc_prev, op=ALU.mult
                        )

                def proj_chunk(ti, ci):
                    """out-proj chunk + residual-1 into out1 (PE+DVE work)."""
                    off, sz, bt, _ = TOK_TILES[ti]
                    pm = psB.tile([128, 512], F32, tag="mm", name="pj", bufs=1)
                    for kc in range(KC):
                        nc.tensor.matmul(
                            pm[:sz],
                            oTb[bt][:, kc, off - bt * N : off - bt * N + sz],
                            wp_sb[:, kc, ci * 512 : (ci + 1) * 512],
                            start=(kc == 0),
                            stop=(kc == KC - 1 and "bp" not in bias_sb),
                        )
                    if "bp" in bias_sb:
                        nc.tensor.matmul(
                            pm[:sz],
                            ones_m[0:1, :sz],
                            bias_sb["bp"][0:1, ci * 512 : (ci + 1) * 512],
                            start=False,
                            stop=True,
                        )
                    x_t2 = p_xr.tile([128, 512], F32, name="x_t2")
                    nc.sync.dma_start(
                        out=x_t2[:sz],
                        in_=x_d[off : off + sz, ci * 512 : (ci + 1) * 512],
                    )
                    nc.vector.scalar_tensor_tensor(
                        out=out1[:sz, ti, ci * 512 : (ci + 1) * 512],
                        in0=pm[:sz],
                        scalar=1.0,
                        in1=x_t2[:sz],
                        op0=ALU.mult,
                        op1=ALU.add,
                    )

                def attn_pair(b, j, fillers):
                    """One head pair (2j, 2j+1); fillers: list of closures to
                    interleave for PE occupancy while ACT grinds the exps."""
                    sb0 = b * N
                    # AV pair accumulator: 3 banks.
                    # cols [0:512]    h0 q-chunk0   (bank 0)
                    # cols [512:577]  h0 q-chunk1   (bank 1, shared group)
                    # cols [577:642]  h1 q-chunk1   (bank 1, shared group)
                    # cols [1024:1536] h1 q-chunk0  (bank 2)
                    # drain deferred normalization mults now: DVE is idle at
                    # pair start, and this keeps them out of the queue ahead
                    # of the av evictions at pair end (which gate the next
                    # pair's first AV matmul on the single-buffered av tile)
                    flush_pending()
                    av = psB.tile([65, 1536], F32, tag="av", name="av", bufs=1)

                    def av_pair(e_kt, kt, ksz):
                        for hi in range(2):
                            h = 2 * j + hi
                            vt = v_att[:ksz, b * 5 + kt, h * 65 : (h + 1) * 65]
                            # q-chunk0 -> own bank
                            c0off = hi * 1024
                            nc.tensor.matmul(
                                av[:, c0off : c0off + 512],
                                vt,
                                e_kt[hi][:ksz, 0:512],
                                start=(kt == 0),
                                stop=(kt == 4),
                            )
                            # q-chunk1 -> shared bank-1 group
                            c1off = 512 + hi * 65
                            nc.tensor.matmul(
                                av[:, c1off : c1off + 65],
                                vt,
                                e_kt[hi][:ksz, 512:577],
                                start=(kt == 0 and hi == 0),
                                stop=(kt == 4 and hi == 1),
                                skip_group_check=True,
                            )

                    # software-pipelined by one kt: S/exp(kt+1) issue ahead of
                    # AV(kt) so the PE never queues behind the ACT exps
                    avq = []
                    for kt in range(5):
                        koff = sb0 + kt * 128
                        ksz = min(128, N - kt * 128)
                        # chunk-major issue order: the two heads' 512-wide S
                        # matmuls are adjacent in the PE queue and target
                        # disjoint row-groups (0-63 / 64-127), so they run
                        # concurrently on the array
                        ps_ss = [
                            psB.tile([128, N], F32, tag="s", name="ps_s", bufs=2)
                            for _ in range(2)
                        ]
                        for q0, qsz in QCHUNKS:
                            for hi in range(2):
                                po = hi * 64
                                nc.tensor.matmul(
                                    ps_ss[hi][:ksz, q0 : q0 + qsz],
                                    kT[po : po + 64, j, koff : koff + ksz],
                                    qT[po : po + 64, j, sb0 + q0 : sb0 + q0 + qsz],
                                    start=True,
                                    stop=True,
                                )
                        e_kt = []
                        for hi in range(2):
                            e_t = p_e.tile([128, N], BF16, tag="E", name="E", bufs=4)
                            nc.scalar.activation(
                                out=e_t[:ksz], in_=ps_ss[hi][:ksz],
                                func=AF.Exp, scale=0.125,
                            )
                            e_kt.append(e_t)
                        avq.append((e_kt, kt, ksz))
                        if kt >= 1:
                            av_pair(*avq.pop(0))
                        if kt == 2 and fillers:
                            fillers.pop(0)()
                    av_pair(*avq.pop(0))
                    # evict unnormalized o + sums FIRST (frees the av banks
                    # for the next pair), then queue the normalization chain
                    s_sbs = []
                    for hi in range(2):
                        po = hi * 64
                        c0off = hi * 1024
                        c1off = 512 + hi * 65
                        nc.vector.tensor_copy(
                            out=oTb[b][po : po + 64, j, 0:512],
                            in_=av[0:64, c0off : c0off + 512],
                        )
                        nc.vector.tensor_copy(
                            out=oTb[b][po : po + 64, j, 512:577],
                            in_=av[0:64, c1off : c1off + 65],
                        )
                        s_sb = p_s.tile([1, N], F32, name="s_sb", bufs=3)
                        nc.vector.tensor_copy(
                            out=s_sb[:, 0:512], in_=av[64:65, c0off : c0off + 512]
                        )
                        nc.vector.tensor_copy(
                            out=s_sb[:, 512:577], in_=av[64:65, c1off : c1off + 65]
                        )
                        s_sbs.append(s_sb)
                    for hi in range(2):
                        po = hi * 64
                        r_sb = p_s.tile([1, N], F32, name="r_sb", bufs=3)
                        nc.vector.reciprocal_approx_fast(out=r_sb, in_=s_sbs[hi])
                        r_bc = p_s.tile([128, N], F32, name="r_bc", bufs=3)
                        nc.gpsimd.partition_broadcast(r_bc, r_sb)
                        pending.append(
                            (oTb[b][po : po + 64, j, :], r_bc[po : po + 64])
                        )

                # b0 attention, no fillers
                for j in range(KC):
                    attn_pair(0, j, [])
                flush_pending()

                # b1 attention with b0 proj interleaved (PE-heavy pieces;
                # LN2 stays out of this window: its Sqrt would force an ACT
                # table switch away from Exp, ~2.7us each way)
                work = []
                for ti in range(5):
                    work.append(lambda ti=ti: proj_chunk(ti, 0))
                    work.append(lambda ti=ti: proj_chunk(ti, 1))
                for j in range(KC):
                    attn_pair(1, j, work)
                flush_pending()
                while work:
                    work.pop(0)()

            # ---- phase 1c: b1 proj + LN2 + transposes (all tiles) ----
            with ExitStack() as ph1c:
                psD = ph1c.enter_context(
                    tc.tile_pool(name="psD", bufs=1, space="PSUM")
                )
                p_xr2 = ph1c.enter_context(tc.tile_pool(name="p_xr2", bufs=4))
                p_ln2b = ph1c.enter_context(tc.tile_pool(name="p_ln2b", bufs=2))
                for ti in range(5, 10):
                    off, sz, bt, _ = TOK_TILES[ti]
                    for ci in range(2):
                        pm = psD.tile([128, 512], F32, tag="mm", name="pj2", bufs=2)
                        for kc in range(KC):
                            nc.tensor.matmul(
                                pm[:sz],
                                oTb[bt][:, kc, off - bt * N : off - bt * N + sz],
                                wp_sb[:, kc, ci * 512 : (ci + 1) * 512],
                                start=(kc == 0),
                                stop=(kc == KC - 1 and "bp" not in bias_sb),
                            )
                        if "bp" in bias_sb:
                            nc.tensor.matmul(
                                pm[:sz],
                                ones_m[0:1, :sz],
                                bias_sb["bp"][0:1, ci * 512 : (ci + 1) * 512],
                                start=False,
                                stop=True,
                            )
                        x_t2 = p_xr2.tile([128, 512], F32, name="x_t2")
                        nc.sync.dma_start(
                            out=x_t2[:sz],
                            in_=x_d[off : off + sz, ci * 512 : (ci + 1) * 512],
                        )
                        nc.vector.scalar_tensor_tensor(
                            out=out1[:sz, ti, ci * 512 : (ci + 1) * 512],
                            in0=pm[:sz],
                            scalar=1.0,
                            in1=x_t2[:sz],
                            op0=ALU.mult,
                            op1=ALU.add,
                        )
                # LN2 + transpose for every token tile (b0's out1 is ready;
                # overlaps the b1 proj above via engine parallelism)
                for ti, (off, sz, _, _) in enumerate(TOK_TILES):
                    h_t2 = ln_tile(p_ln2b, out1[:, ti, :], sz, "2")
                    transpose_into(psD, h_t2, sz, h2T, off, tag="tr")

        # ---------------- phase 2: MLP ----------------
        with ExitStack() as ph2:
            psC = ph2.enter_context(tc.tile_pool(name="psC", bufs=1, space="PSUM"))
            p_gu = ph2.enter_context(tc.tile_pool(name="p_gu", bufs=1))
            p_w2r = ph2.enter_context(tc.tile_pool(name="p_w2r", bufs=1))
            p_w1s = ph2.enter_context(tc.tile_pool(name="p_w1s", bufs=3))
            p_sg = ph2.enter_context(tc.tile_pool(name="p_sg", bufs=2))
            p_res = ph2.enter_context(tc.tile_pool(name="p_res", bufs=4))

            guT = p_gu.tile([128, MC, T], BF16, name="guT")
            w2sb = p_w2r.tile([128, MC, C], BF16, name="w2sb")

            # fc1 + fast-gelu; W2 prefetch rides the gpsimd DGE queue
            for m in range(MC):
                w_t = p_w1s.tile([128, C], BF16, name="w1_t")
                nc.sync.dma_start(out=w_t, in_=w1_d[m])
                nc.gpsimd.dma_start(out=w2sb[:, m, :], in_=w2_d[m])
                for c0, csz in TCHUNKS:
                    pm = psC.tile(
                        [128, 512], F32,
                        tag="mm" if (m & 1) == 0 else "tr",
                        name="f1", bufs=2,
                    )
                    for kc in range(KC):
                        nc.tensor.matmul(
                            pm[:, :csz],
                            w_t[:, kc * 128 : (kc + 1) * 128],
                            h2T[:, kc, c0 : c0 + csz],
                            start=(kc == 0),
                            stop=(kc == KC - 1 and "b1" not in bias_sb),
                        )
                    if "b1" in bias_sb:
                        nc.tensor.matmul(
                            pm[:, :csz],
                            bias_sb["b1"][0:1, m * 128 : (m + 1) * 128],
                            ones_r[0:1, :csz],
                            start=False,
                            stop=True,
                        )
                    sg = p_sg.tile([128, 512], F32, name="sg")
                    nc.scalar.activation(
                        out=sg[:, :csz], in_=pm[:, :csz], func=AF.Sigmoid, scale=1.702
                    )
                    nc.vector.tensor_tensor(
                        out=guT[:, m, c0 : c0 + csz],
                        in0=pm[:, :csz],
                        in1=sg[:, :csz],
                        op=ALU.mult,
                    )

            # fc2 + residual-2, straight to DRAM
            for ti, (off, sz, _, _) in enumerate(TOK_TILES):
                pms = [
                    psC.tile([128, 512], F32, tag="mm", name="f2a", bufs=2),
                    psC.tile([128, N], F32, tag="wide", name="f2b", bufs=2),
                ]
                for hc in range(MC):
                    for half in range(2):
                        nc.tensor.matmul(
                            pms[half][:sz, :512],
                            guT[:, hc, off : off + sz],
                            w2sb[:, hc, half * 512 : half * 512 + 512],
                            start=(hc == 0),
                            stop=(hc == MC - 1 and "b2" not in bias_sb),
                        )
                for half in range(2):
                    h0 = half * 512
                    if "b2" in bias_sb:
                        nc.tensor.matmul(
                            pms[half][:sz, :512],
                            ones_m[0:1, :sz],
                            bias_sb["b2"][0:1, h0 : h0 + 512],
                            start=False,
                            stop=True,
                        )
                    res = p_res.tile([128, 512], F32, name="res")
                    nc.vector.scalar_tensor_tensor(
                        out=res[:sz],
                        in0=pms[half][:sz, :512],
                        scalar=1.0,
                        in1=out1[:sz, ti, h0 : h0 + 512],
                        op0=ALU.mult,
                        op1=ALU.add,
                    )
                    nc.sync.dma_start(
                        out=out_d[off : off + sz, h0 : h0 + 512], in_=res[:sz]
                    )

    nc.compile()
    return nc


def _prepare(inputs):
    """Host-side: fold norms into weights, transpose/block, shard x."""
    f = lambda k: np.asarray(inputs[k], dtype=np.float32)
    x = f("x")
    g1, b1ln = f("norm1_g"), f("norm1_b")
    g2, b2ln = f("norm2_g"), f("norm2_b")
    Wq, Wk, Wv, Wp = f("Wq"), f("Wk"), f("Wv"), f("Wp")
    W1, W2 = f("W1"), f("W2")
    bp, b1, b2 = f("bp"), f("b1"), f("b2")

    wq = _block_lhs((Wq * g1[None, :]).T, KC)
    wk = _block_lhs((Wk * g1[None, :]).T, KC)
    wv = np.ascontiguousarray((Wv * g1[None, :]).T.reshape(KC, 128, C))
    wp = np.ascontiguousarray(Wp.T.reshape(KC, 128, C))
    w1 = _block_lhs((W1 * g2[None, :]).T, MC)
    w2 = np.ascontiguousarray(W2.T.reshape(MC, 128, C))

    biases = {
        "bq": Wq @ b1ln,
        "bk": Wk @ b1ln,
        "bv": Wv @ b1ln,
        "bp": bp,
        "b1": W1 @ b2ln + b1,
        "b2": b2,
    }
    use_bias = {k: bool(np.any(v != 0)) for k, v in biases.items()}

    weights = {
        "wq": _bf(wq),
        "wk": _bf(wk),
        "wv": _bf(wv),
        "wp": _bf(wp),
        "w1": _bf(w1),
        "w2": _bf(w2),
    }
    for k, v in biases.items():
        if use_bias[k]:
            weights[k] = _bf(v.reshape(1, -1))

    xs = x.reshape(NCORES, T, C)
    in_maps = [dict(weights, x=np.ascontiguousarray(xs[i])) for i in range(NCORES)]
    return in_maps, use_bias


_CACHE = {}


def _get_program(use_bias):
    key = tuple(sorted(use_bias.items()))
    if key not in _CACHE:
        _CACHE[key] = _build(use_bias)
    return _CACHE[key]


def run(inputs, trace=False, **kw):
    in_maps, use_bias = _prepare(inputs)
    nc = _get_program(use_bias)
    res = run_bass_kernel_spmd(
        nc, in_maps, core_ids=list(range(NCORES)), trace=trace, **kw
    )
    out = np.concatenate([res.results[i]["out"] for i in range(NCORES)], axis=0)
    return np.ascontiguousarray(out.astype(np.float32)), res


def kernel(**inputs):
    out, _ = run(inputs, trace=False)
    return out
